# revision 35
# baseline (speedup 1.0000x reference)
"""Trainium2 Bass kernel for nn_Abcnn2Portion (ABCNN-2 attention pooling).

Shapes (hardcoded): B=16, N=259 (L=256 + W-1=3), H=128, W=4, EPS=1e-6.
Reference:
    att[b,i,j] = 1 / (1 + sqrt(||x1[b,0,j,:] - x2[b,0,i,:]||^2 + EPS))
    x1_a[b,j] = sum_i att[b,i,j];  x2_a[b,i] = sum_j att[b,i,j]
    out_t[b,0,l,:] = sum_{k=0..3} x_t[b,0,l+k,:] * a_t[b,l+k],  l in [0,256)
Returns (out1, out2), each (16,1,256,128) fp32.

Strategy: data-parallel over batch, 2 batches per core across 8 cores.
Per batch:
  - pairwise squared distances via PE matmuls accumulated in PSUM
    (s2 = -2*x2.x1 + sq1[j] + sq2[i], all via bf16 matmuls; the d-major
    layout comes from fp32 PE transposes whose psum->sbuf copy casts to
    bf16 and folds the -2 scale for x2);
  - sqrt on the scalar engine (EPS via a per-partition bias tile);
  - att = 1/(1+e) fused with the row-sum in a single custom DVE op
    (bit-trick seed + one Newton step, zero-mean ~0.17% elementwise,
    averages out in the sums);
  - column sums via ones-matmul;
  - the W=4 sliding-window weighted pooling as a banded-matrix matmul
    (float32r), batch-split so each batch's output DMA drains while the
    other batch still computes.

Measured on trn2 (8 cores): ~34 us NEFF execution, rel err ~1.0e-3 vs
the fp32 jax reference. Roughly 7 us of that is fixed NRT preamble
(entry barrier + per-engine IRAM loads) and ~10 us the Tile kernel-tail
drain/barrier/sem-clear sequence; the PE on this terminal is capped at
1.2 GHz (power profile), which sets the matmul costs.
"""

import numpy as np

import concourse.bass as bass
import concourse.tile as tile
from concourse import mybir
from concourse.bass_utils import run_bass_kernel_spmd

# --------------------------------------------------------------------------
# Custom DVE op: out = approx(1/(1 + x)), accum_out = sum(out, free axis).
# --------------------------------------------------------------------------
import concourse.dve_ops as dve_ops
from concourse.dve_spec import Spec, Src0, C0, C1, One, AluOp, Bin, lower, _has_src1
from concourse.dve_ops import DveOp, OPS
from concourse.dve_uop import DveOpSpec

_S = Src0 + One
_nt = Bin(AluOp.BITWISE_NOT, _S, _S)
_y0 = _nt * C0
_BODY = _y0 * (C1 - _S * _y0)


def _recip_ref(in0, in1, s0, s1, imm2):
    S = (in0.astype(np.float32) + np.float32(1.0)).astype(np.float32)
    nt = (~S.view(np.int32)).view(np.float32)
    y0 = nt * np.float32(s0)
    return y0 * (np.float32(s1) - S * y0)


def _register_recip_op():
    name = "ADD1_RECIP_SUM_ANT"
    for existing in OPS:
        if existing.name == name:
            return existing
    spec = Spec(body=_BODY, accum=AluOp.ADD, reference=_recip_ref)
    op = DveOp(name, spec, subdim=False, uops_sha={})
    OPS.append(op)
    dve_ops._SUB_OPCODE_FOR_NAME[name] = dve_ops._CUSTOM_DVE_ROW_BASE + len(OPS) - 1
    for ver in ("v3", "v4"):
        op.uops_sha[ver] = DveOpSpec(
            name=name,
            opcode=dve_ops.get_dve_sub_opcode(name),
            uops=lower(spec, ver=ver),
            rd1_en=_has_src1(spec),
        ).sha(ver)
    return op


RECIP_OP = _register_recip_op()
RECIP_C0 = -0.23549792
RECIP_C1 = 2.0017324

# --------------------------------------------------------------------------
# Problem constants
# --------------------------------------------------------------------------
B, L, W, H = 16, 256, 4, 128
N = L + W - 1  # 259
EPS = 1e-6
NCORES = 8
BPC = B // NCORES  # batches per core = 2

f32 = mybir.dt.float32
f32r = mybir.dt.float32r
bf16 = mybir.dt.bfloat16
AF = mybir.ActivationFunctionType
ALU = mybir.AluOpType

# i-chunk decomposition of N=259: two full 128-partition chunks + 3 leftover
CHUNKS = [(0, 128), (128, 128), (256, 3)]
# pad the shared free dim to keep matmul moving dims even (f32r requires it;
# harmless for bf16). The pad column is garbage and excluded where it matters.
NP = 260


def _host_consts():
    """DMA-able constant tables: identity, band main, band boundary."""
    ident = np.eye(128, dtype=np.float32)
    band = np.zeros((128, 128), dtype=np.float32)  # band[n, l] = 1 if l<=n<=l+3
    for n in range(128):
        lo = max(0, n - (W - 1))
        band[n, lo : n + 1] = 1.0
    bandb = np.zeros((128, 128), dtype=np.float32)
    for r in range(W - 1):  # boundary rows n = 128+r relative to l in [0,128)
        bandb[r, 125 + r : 128] = 1.0
    return np.stack([ident, band, bandb])  # (3, 128, 128)


def build_nc():
    nc = bass.Bass()
    _orig_dab = tile.TileContext._drain_and_barrier

    def _light_dab(self, tick_clock, wait_clock):
        import bass_rust as _br
        _vc_mod = __import__('concourse.vector_clock', fromlist=['ScopedClock'])
        drain_inst = self.nc.sync.drain()
        gvc = tick_clock.global_clock
        dvc = _br.VectorClock([0] * _br.N_PROCS)
        for p in range(11, _br.N_PROCS):  # DMASW0..7, DMAHW0..7
            t = gvc[p]
            if t > 0:
                dvc.require_at_least(p, t)
        wait_clock.add_sem_waits(
            drain_inst.ins, _vc_mod.ScopedClock({None: dvc})
        )
        self.nc.all_engine_barrier()
        assert self.sems is not None
        popped = self.nc._tile_sem_poison_stack.pop()
        assert popped is self._sem_poison
        self.nc.clear_and_free_semaphores(list(self.sems.allocated().values()))
        self.nc.all_engine_barrier(sem_only=True)

    tile.TileContext._drain_and_barrier = _light_dab
    try:
        _build_body(nc)
    finally:
        tile.TileContext._drain_and_barrier = _orig_dab
    # TRN2 allows at most 1 sem wait per instruction (2 on EventSemaphore);
    # Tile can attach more — split them like Bacc.compile does, then encode
    # InstISA subclasses (custom DVE ops) to raw ISA bytes.
    import bass_rust
    from concourse import mybir as _mybir
    bass_rust.generate_event_semaphores(nc)
    _mybir.codegen_inst_isa_subclasses(nc)
    return nc


def _build_body(nc):
    x1_in = nc.dram_tensor("x1", [BPC, 1, N, H], f32, kind="ExternalInput")
    x2_in = nc.dram_tensor("x2", [BPC, 1, N, H], f32, kind="ExternalInput")
    out1_d = nc.dram_tensor("out1", [BPC, 1, L, H], f32, kind="ExternalOutput")
    out2_d = nc.dram_tensor("out2", [BPC, 1, L, H], f32, kind="ExternalOutput")

    xin = {0: x1_in, 1: x2_in}
    outd = {0: out1_d, 1: out2_d}

    with tile.TileContext(nc) as tc:
        with (
            tc.tile_pool(name="singles", bufs=1) as singles,
            tc.tile_pool(name="work", bufs=1) as work,
            tc.tile_pool(name="epool", bufs=6) as epool,
            tc.tile_pool(name="attpool", bufs=6) as attpool,
            tc.tile_pool(name="tp_ps", bufs=1, space="PSUM") as tp_ps,
            tc.tile_pool(name="gram_ps", bufs=4, space="PSUM") as gram_ps,
            tc.tile_pool(name="row_ps", bufs=1, space="PSUM") as row_ps,
            tc.tile_pool(name="acol_ps", bufs=1, space="PSUM") as acol_ps,
            tc.tile_pool(name="band_ps", bufs=1, space="PSUM") as band_ps,
        ):
            # ---- dependency-free startup: act-table trigger ----
            epsb = singles.tile([128, 1], f32, tag="epsb")
            escr = singles.tile([128, 1], f32, tag="escr")
            nc.vector.memset(epsb[:, :], EPS)
            # tiny Sqrt with no data deps: starts the 2.7us ACT table load
            # (sqrt_and_others, which also contains Copy/Identity/Square)
            # at t~0 instead of mid-kernel.
            nc.scalar.activation(escr[:, :], epsb[:, :], AF.Sqrt)

            # ---- inputs first: big x loads on the two HWDGE rings ----
            x_nd, x_l = {}, {}
            for t in (0, 1):
                x_nd[t] = work.tile(
                    [128, BPC, 2, H], f32, tag=f"x{t}nd", name=f"x{t}nd"
                )
                x_l[t] = work.tile([3, BPC, H], f32, tag=f"x{t}l", name=f"x{t}l")
                dma_eng = nc.sync if t == 0 else nc.scalar
                for b in range(BPC):
                    dma_eng.dma_start(
                        out=x_nd[t][:, b, :, :],
                        in_=xin[t][b, 0, 0:L, :].rearrange(
                            "(c p) h -> p c h", c=2
                        ),
                    )
            # small leftover rows on the sync ring (keep the scalar/ACT
            # queue free for activations)
            for t in (0, 1):
                for b in range(BPC):
                    nc.sync.dma_start(
                        out=x_l[t][:, b, :], in_=xin[t][b, 0, L:N, :]
                    )

            # ---- constants, generated on-chip (no DMA) ----
            band32 = singles.tile([128, 128], f32, tag="band32")
            bandb32 = singles.tile([3, 128], f32, tag="bandb32")
            ident32 = singles.tile([128, 128], f32, tag="ident32")
            band = singles.tile([128, 128], f32r, tag="band")
            bandb = singles.tile([3, 128], f32r, tag="bandb")
            ones16 = singles.tile([128, NP], bf16, tag="ones16")
            ones_f = singles.tile([128, NP], f32, tag="ones_f")
            ones32 = singles.tile([1, 1], f32, tag="ones32")
            nc.gpsimd.memset(ones_f[:, :], 1.0)
            nc.vector.tensor_copy(ones16[:, :], ones_f[:, :])
            quart16 = singles.tile([128, NP], bf16, tag="quart16")
            nc.vector.tensor_scalar(
                out=quart16[:, :], in0=ones_f[:, :],
                scalar1=0.25, scalar2=None, op0=ALU.mult,
            )
            nc.gpsimd.memset(ones32[:, :], 1.0)
            # ident[p, l] = (p - l == 0)
            nc.gpsimd.affine_select(
                out=ident32[:, :], in_=ones_f[:, 0:128],
                pattern=[[-1, 128]], compare_op=ALU.is_equal, fill=0.0,
                base=0, channel_multiplier=1,
            )
            # band[n, l] = (n - l >= 0) & (n - l - 3 <= 0)
            nc.gpsimd.affine_select(
                out=band32[:, :], in_=ones_f[:, 0:128],
                pattern=[[-1, 128]], compare_op=ALU.is_ge, fill=0.0,
                base=0, channel_multiplier=1,
            )
            nc.gpsimd.affine_select(
                out=band32[:, :], in_=band32[:, :],
                pattern=[[1, 128]], compare_op=ALU.is_ge, fill=0.0,
                base=3, channel_multiplier=-1,
            )
            # bandb[r, l] = (l - 125 - r >= 0)
            nc.gpsimd.affine_select(
                out=bandb32[:, :], in_=ones_f[0:3, 0:128],
                pattern=[[1, 128]], compare_op=ALU.is_ge, fill=0.0,
                base=-125, channel_multiplier=-1,
            )
            nc.vector.tensor_copy(band[:, :], band32[:, :])
            nc.vector.tensor_copy(bandb[:, :], bandb32[:, :])

            # ---- d-major bf16 layout via fp32 PE transposes; the
            # psum->sbuf copy performs the bf16 cast (and the -2 scale for
            # x2's Gram operand).
            x_dn16 = {0: {}, 1: {}}
            for t in (0, 1):
                for b in range(BPC):
                    dn = work.tile(
                        [128, NP], bf16, tag=f"x{t}dn{b}", name=f"x{t}dn{b}"
                    )
                    tpp = tp_ps.tile([128, N], f32, tag="tp")
                    for c in (0, 1):
                        nc.tensor.transpose(
                            tpp[:, c * 128 : (c + 1) * 128],
                            x_nd[t][:, b, c, :],
                            ident32[:, :],
                        )
                    nc.tensor.transpose(
                        tpp[:, 256:259], x_l[t][:, b, :], ident32[0:3, 0:3]
                    )
                    nc.vector.tensor_scalar(
                        out=dn[:, 0:N], in0=tpp[:, 0:N],
                        scalar1=1.0 if t == 0 else -2.0,
                        scalar2=None, op0=ALU.mult,
                    )
                    x_dn16[t][b] = dn

            # xsq[t][b] = x_t_dn16^2 (bf16) on the DVE (2x bf16 mode); x2's
            # dn is scaled by -2 so its square is 4*x2^2 — compensated by
            # using 0.25-valued "ones" in its broadcast matmul.
            xsq = {0: {}, 1: {}}
            for t in (0, 1):
                for b in range(BPC):
                    sq = work.tile(
                        [128, NP], bf16, tag=f"xsq{t}{b}", name=f"xsq{t}{b}"
                    )
                    nc.scalar.activation(
                        sq[:, 0:N], x_dn16[t][b][:, 0:N], AF.Square,
                    )
                    xsq[t][b] = sq

            # ---- per-chunk sq2 + EPS (sqrt bias) ----
            a_cols = {0: {}, 1: {}}  # (128, 4) sbuf; cols = chunks
            for t in (0, 1):
                for b in range(BPC):
                    a_cols[t][b] = work.tile(
                        [128, 4], f32, tag=f"a{t}c{b}", name=f"a{t}c{b}"
                    )

            # wx tiles (weighted inputs for the pooling matmul); c-slot 2
            # rows 0..2 hold the weighted leftover rows n=256..258
            wx = {}
            for t in (0, 1):
                wx[t] = work.tile(
                    [128, BPC, 3, H], f32r, tag=f"wx{t}", name=f"wx{t}"
                )

            def emit_wx(t, b):
                for c in (0, 1):
                    nc.vector.tensor_scalar(
                        out=wx[t][:, b, c, :], in0=x_nd[t][:, b, c, :],
                        scalar1=a_cols[t][b][:, c : c + 1],
                        scalar2=None, op0=ALU.mult,
                    )
                nc.vector.tensor_scalar(
                    out=wx[t][0:3, b, 2, :], in0=x_l[t][:, b, :],
                    scalar1=a_cols[t][b][0:3, 2:3],
                    scalar2=None, op0=ALU.mult,
                )

            # ---- attention chunks ----
            x1row_sb = {}
            for b in range(BPC):
                rowp = row_ps.tile([1, NP], f32, tag="x1row")
                for ci, (i0, P) in enumerate(CHUNKS):
                    g = gram_ps.tile([128, NP], f32, tag="gram")
                    # -2 * x2[:,i] . x1[:,j]
                    nc.tensor.matmul(
                        g[0:P, :],
                        x_dn16[1][b][:, i0 : i0 + P],
                        x_dn16[0][b][:, :],
                        start=True, stop=False,
                    )
                    # + sq1[j] broadcast over i  (ones^T @ xsq1)
                    nc.tensor.matmul(
                        g[0:P, :],
                        ones16[:, 0:P],
                        xsq[0][b][:, :],
                        start=False, stop=False,
                    )
                    # + sq2[i] broadcast over j: xsq2 holds (2*x2)^2, the
                    # 0.25-valued moving operand recovers x2^2.
                    nc.tensor.matmul(
                        g[0:P, :],
                        xsq[1][b][:, i0 : i0 + P],
                        quart16[:, :],
                        start=False, stop=True,
                    )
                    # e = sqrt(psum + EPS)
                    e = epool.tile([128, NP], f32, tag="e")
                    nc.scalar.activation(
                        e[0:P, :], g[0:P, :], AF.Sqrt,
                        bias=epsb[0:P, 0:1],
                    )
                    # att = 1/(1+e) approx; accum -> x2_a column
                    att = attpool.tile([128, NP], bf16, tag="att")
                    nc.vector._custom_dve(
                        RECIP_OP,
                        out=att[0:P, 0:N], in0=e[0:P, 0:N],
                        s0=RECIP_C0, s1=RECIP_C1,
                        accum_out=a_cols[1][b][0:P, ci : ci + 1],
                    )
                    # x1_a row accumulation: ones_col^T @ att
                    nc.tensor.matmul(
                        rowp[:, :],
                        ones16[0:P, 0:1],
                        att[0:P, :],
                        start=(ci == 0), stop=(ci == 2),
                    )
                row_sb = work.tile([1, NP], f32, tag="x1row_sb", name=f"x1row{b}")
                nc.vector.tensor_copy(row_sb[:, :], rowp[:, :])
                x1row_sb[b] = row_sb
                # x2 weights for this batch are complete: start wx2 now
                emit_wx(1, b)

            # x1_a row -> per-partition columns via tiny K=1 matmuls
            for b in range(BPC):
                ac = acol_ps.tile([128, 4], f32, tag="acolp")
                for ci, (i0, P) in enumerate(CHUNKS):
                    nc.tensor.matmul(
                        ac[0:P, ci : ci + 1],
                        x1row_sb[b][:, i0 : i0 + P],
                        ones32[0:1, 0:1],
                        start=True, stop=True,
                    )
                nc.vector.tensor_copy(a_cols[0][b][:, :], ac[:, :])
                emit_wx(0, b)

            # ---- weighted sliding-window pooling via banded matmul (f32r),
            # batch-split so batch 0's output drains while batch 1 computes ----
            for b in range(BPC):
                for t in (0, 1):
                    bp = band_ps.tile([128, 2, BPC, H], f32, tag="bandp")
                    # main rows: out[l, lc, h] += band[n, l] * wx[n, b, lc, h]
                    nc.tensor.matmul(
                        bp[:, :, b, :],
                        band[:, :],
                        wx[t][:, b, 0:2, :],
                        start=True, stop=False,
                    )
                    # boundary: lc=0 <- n=128..130 (c=1), lc=1 <- n=256..258
                    nc.tensor.matmul(
                        bp[:, :, b, :],
                        bandb[0:3, :],
                        wx[t][0:3, b, 1:3, :],
                        start=False, stop=True,
                    )
                    osb = work.tile(
                        [128, 2, H], f32, tag=f"osb{t}{b}", name=f"osb{t}{b}"
                    )
                    if t == 0:
                        nc.scalar.copy(osb[:, :, :], bp[:, :, b, :])
                    else:
                        nc.vector.tensor_copy(osb[:, :, :], bp[:, :, b, :])
                    (nc.sync if t == 0 else nc.scalar).dma_start(
                        out=outd[t][b, 0, :, :].rearrange(
                            "(lc p) h -> p lc h", lc=2
                        ),
                        in_=osb[:, :, :],
                    )
    # TRN2 allows at most 1 sem wait per instruction (2 on EventSemaphore);
    # Tile can attach more — split them like Bacc.compile does, then encode
    # InstISA subclasses (custom DVE ops) to raw ISA bytes.
    import bass_rust
    from concourse import mybir as _mybir
    bass_rust.generate_event_semaphores(nc)
    _mybir.codegen_inst_isa_subclasses(nc)
    return nc


_NC_CACHE = {}


def _get_nc():
    if "nc" not in _NC_CACHE:
        _NC_CACHE["nc"] = build_nc()
    return _NC_CACHE["nc"]


def _run(x1, x2, **kwargs):
    x1 = np.ascontiguousarray(np.asarray(x1), dtype=np.float32)
    x2 = np.ascontiguousarray(np.asarray(x2), dtype=np.float32)
    nc = _get_nc()
    core_ids = list(range(NCORES))
    in_maps = [
        {
            "x1": x1[c * BPC : (c + 1) * BPC],
            "x2": x2[c * BPC : (c + 1) * BPC],
        }
        for c in core_ids
    ]
    br = run_bass_kernel_spmd(nc, in_maps, core_ids, **kwargs)
    out1 = np.concatenate([r["out1"] for r in br.results], axis=0)
    out2 = np.concatenate([r["out2"] for r in br.results], axis=0)
    return (out1, out2), br


def kernel(x1, x2):
    (out1, out2), _ = _run(x1, x2)
    return (out1, out2)


if __name__ == "__main__":
    rng = np.random.default_rng(0)
    x1 = rng.standard_normal((B, 1, N, H)).astype(np.float32)
    x2 = rng.standard_normal((B, 1, N, H)).astype(np.float32)
    o1, o2 = kernel(x1, x2)
    print("out shapes:", o1.shape, o2.shape)


# revision 37
# speedup vs baseline: 1.2223x; 1.2223x over previous
"""Trainium2 Bass kernel for nn_Abcnn2Portion (ABCNN-2 attention pooling).

Shapes (hardcoded): B=16, N=259 (L=256 + W-1=3), H=128, W=4, EPS=1e-6.
Reference:
    att[b,i,j] = 1 / (1 + sqrt(||x1[b,0,j,:] - x2[b,0,i,:]||^2 + EPS))
    x1_a[b,j] = sum_i att[b,i,j];  x2_a[b,i] = sum_j att[b,i,j]
    out_t[b,0,l,:] = sum_{k=0..3} x_t[b,0,l+k,:] * a_t[b,l+k],  l in [0,256)
Returns (out1, out2), each (16,1,256,128) fp32.

Strategy: data-parallel over batch, 2 batches per core across 8 cores.
Per batch:
  - pairwise squared distances via PE matmuls accumulated in PSUM
    (s2 = -2*x2.x1 + sq1[j] + sq2[i], all via bf16 matmuls; the d-major
    layout comes from fp32 PE transposes whose psum->sbuf copy casts to
    bf16 and folds the -2 scale for x2);
  - sqrt on the scalar engine (EPS via a per-partition bias tile);
  - att = 1/(1+e) fused with the row-sum in a single custom DVE op
    (bit-trick seed + one Newton step, zero-mean ~0.17% elementwise,
    averages out in the sums);
  - column sums via ones-matmul;
  - the W=4 sliding-window weighted pooling as a banded-matrix matmul
    (float32r), batch-split so each batch's output DMA drains while the
    other batch still computes.

Measured on trn2 (8 cores): ~34 us NEFF execution, rel err ~1.0e-3 vs
the fp32 jax reference. Roughly 7 us of that is fixed NRT preamble
(entry barrier + per-engine IRAM loads) and ~10 us the Tile kernel-tail
drain/barrier/sem-clear sequence; the PE on this terminal is capped at
1.2 GHz (power profile), which sets the matmul costs.
"""

import numpy as np

import concourse.bass as bass
import concourse.tile as tile
from concourse import mybir
from concourse.bass_utils import run_bass_kernel_spmd

# --------------------------------------------------------------------------
# Custom DVE op: out = approx(1/(1 + x)), accum_out = sum(out, free axis).
# --------------------------------------------------------------------------
import concourse.dve_ops as dve_ops
from concourse.dve_spec import Spec, Src0, C0, C1, One, AluOp, Bin, lower, _has_src1
from concourse.dve_ops import DveOp, OPS
from concourse.dve_uop import DveOpSpec

_S = Src0 + One
_nt = Bin(AluOp.BITWISE_NOT, _S, _S)
_y0 = _nt * C0
_BODY = _y0 * (C1 - _S * _y0)


def _recip_ref(in0, in1, s0, s1, imm2):
    S = (in0.astype(np.float32) + np.float32(1.0)).astype(np.float32)
    nt = (~S.view(np.int32)).view(np.float32)
    y0 = nt * np.float32(s0)
    return y0 * (np.float32(s1) - S * y0)


def _register_recip_op():
    name = "ADD1_RECIP_SUM_ANT"
    for existing in OPS:
        if existing.name == name:
            return existing
    spec = Spec(body=_BODY, accum=AluOp.ADD, reference=_recip_ref)
    op = DveOp(name, spec, subdim=False, uops_sha={})
    OPS.append(op)
    dve_ops._SUB_OPCODE_FOR_NAME[name] = dve_ops._CUSTOM_DVE_ROW_BASE + len(OPS) - 1
    for ver in ("v3", "v4"):
        op.uops_sha[ver] = DveOpSpec(
            name=name,
            opcode=dve_ops.get_dve_sub_opcode(name),
            uops=lower(spec, ver=ver),
            rd1_en=_has_src1(spec),
        ).sha(ver)
    return op


RECIP_OP = _register_recip_op()
RECIP_C0 = -0.23549792
RECIP_C1 = 2.0017324

# --------------------------------------------------------------------------
# Problem constants
# --------------------------------------------------------------------------
B, L, W, H = 16, 256, 4, 128
N = L + W - 1  # 259
EPS = 1e-6
NCORES = 8
BPC = B // NCORES  # batches per core = 2

f32 = mybir.dt.float32
f32r = mybir.dt.float32r
bf16 = mybir.dt.bfloat16
AF = mybir.ActivationFunctionType
ALU = mybir.AluOpType

# i-chunk decomposition of N=259: two full 128-partition chunks + 3 leftover
CHUNKS = [(0, 128), (128, 128), (256, 3)]
# pad the shared free dim to keep matmul moving dims even (f32r requires it;
# harmless for bf16). The pad column is garbage and excluded where it matters.
NP = 260


def _host_consts():
    """DMA-able constant tables: identity, band main, band boundary."""
    ident = np.eye(128, dtype=np.float32)
    band = np.zeros((128, 128), dtype=np.float32)  # band[n, l] = 1 if l<=n<=l+3
    for n in range(128):
        lo = max(0, n - (W - 1))
        band[n, lo : n + 1] = 1.0
    bandb = np.zeros((128, 128), dtype=np.float32)
    for r in range(W - 1):  # boundary rows n = 128+r relative to l in [0,128)
        bandb[r, 125 + r : 128] = 1.0
    return np.stack([ident, band, bandb])  # (3, 128, 128)


def build_nc():
    nc = bass.Bass()
    _orig_dab = tile.TileContext._drain_and_barrier

    def _light_dab(self, tick_clock, wait_clock):
        import bass_rust as _br
        _vc_mod = __import__('concourse.vector_clock', fromlist=['ScopedClock'])
        drain_inst = self.nc.sync.drain()
        gvc = tick_clock.global_clock
        dvc = _br.VectorClock([0] * _br.N_PROCS)
        for p in range(11, _br.N_PROCS):  # DMASW0..7, DMAHW0..7
            t = gvc[p]
            if t > 0:
                dvc.require_at_least(p, t)
        wait_clock.add_sem_waits(
            drain_inst.ins, _vc_mod.ScopedClock({None: dvc})
        )
        self.nc.all_engine_barrier(sem_only=True)
        assert self.sems is not None
        popped = self.nc._tile_sem_poison_stack.pop()
        assert popped is self._sem_poison
        self.nc.clear_and_free_semaphores(list(self.sems.allocated().values()))
        self.nc.all_engine_barrier(sem_only=True)

    tile.TileContext._drain_and_barrier = _light_dab
    try:
        _build_body(nc)
    finally:
        tile.TileContext._drain_and_barrier = _orig_dab
    # TRN2 allows at most 1 sem wait per instruction (2 on EventSemaphore);
    # Tile can attach more — split them like Bacc.compile does, then encode
    # InstISA subclasses (custom DVE ops) to raw ISA bytes.
    import bass_rust
    from concourse import mybir as _mybir
    bass_rust.generate_event_semaphores(nc)
    _mybir.codegen_inst_isa_subclasses(nc)
    return nc


def _build_body(nc):
    x1_in = nc.dram_tensor("x1", [BPC, 1, N, H], f32, kind="ExternalInput")
    x2_in = nc.dram_tensor("x2", [BPC, 1, N, H], f32, kind="ExternalInput")
    out1_d = nc.dram_tensor("out1", [BPC, 1, L, H], f32, kind="ExternalOutput")
    out2_d = nc.dram_tensor("out2", [BPC, 1, L, H], f32, kind="ExternalOutput")

    xin = {0: x1_in, 1: x2_in}
    outd = {0: out1_d, 1: out2_d}

    with tile.TileContext(nc) as tc:
        with (
            tc.tile_pool(name="singles", bufs=1) as singles,
            tc.tile_pool(name="work", bufs=1) as work,
            tc.tile_pool(name="epool", bufs=6) as epool,
            tc.tile_pool(name="attpool", bufs=6) as attpool,
            tc.tile_pool(name="tp_ps", bufs=2, space="PSUM") as tp_ps,
            tc.tile_pool(name="gram_ps", bufs=3, space="PSUM") as gram_ps,
            tc.tile_pool(name="row_ps", bufs=1, space="PSUM") as row_ps,
            tc.tile_pool(name="acol_ps", bufs=1, space="PSUM") as acol_ps,
            tc.tile_pool(name="band_ps", bufs=1, space="PSUM") as band_ps,
        ):
            # ---- dependency-free startup: act-table trigger ----
            epsb = singles.tile([128, 1], f32, tag="epsb")
            escr = singles.tile([128, 1], f32, tag="escr")
            nc.vector.memset(epsb[:, :], EPS)
            # tiny Sqrt with no data deps: starts the 2.7us ACT table load
            # (sqrt_and_others, which also contains Copy/Identity/Square)
            # at t~0 instead of mid-kernel.
            nc.scalar.activation(escr[:, :], epsb[:, :], AF.Sqrt)

            # ---- inputs first: big x loads on the two HWDGE rings ----
            x_nd, x_l = {}, {}
            for t in (0, 1):
                x_nd[t] = work.tile(
                    [128, BPC, 2, H], f32, tag=f"x{t}nd", name=f"x{t}nd"
                )
                x_l[t] = work.tile([3, BPC, H], f32, tag=f"x{t}l", name=f"x{t}l")
                dma_eng = nc.sync if t == 0 else nc.scalar
                for b in range(BPC):
                    dma_eng.dma_start(
                        out=x_nd[t][:, b, :, :],
                        in_=xin[t][b, 0, 0:L, :].rearrange(
                            "(c p) h -> p c h", c=2
                        ),
                    )
            # small leftover rows on the sync ring (keep the scalar/ACT
            # queue free for activations)
            for t in (0, 1):
                for b in range(BPC):
                    nc.sync.dma_start(
                        out=x_l[t][:, b, :], in_=xin[t][b, 0, L:N, :]
                    )

            # ---- constants, generated on-chip (no DMA) ----
            band32 = singles.tile([128, 128], f32, tag="band32")
            bandb32 = singles.tile([3, 128], f32, tag="bandb32")
            ident32 = singles.tile([128, 128], f32, tag="ident32")
            band = singles.tile([128, 128], f32r, tag="band")
            bandb = singles.tile([3, 128], f32r, tag="bandb")
            ones16 = singles.tile([128, NP], bf16, tag="ones16")
            ones_f = singles.tile([128, NP], f32, tag="ones_f")
            ones32 = singles.tile([1, 1], f32, tag="ones32")
            nc.gpsimd.memset(ones_f[:, :], 1.0)
            nc.vector.tensor_copy(ones16[:, :], ones_f[:, :])
            quart16 = singles.tile([128, NP], bf16, tag="quart16")
            nc.vector.tensor_scalar(
                out=quart16[:, :], in0=ones_f[:, :],
                scalar1=0.25, scalar2=None, op0=ALU.mult,
            )
            nc.gpsimd.memset(ones32[:, :], 1.0)
            # ident[p, l] = (p - l == 0)
            nc.gpsimd.affine_select(
                out=ident32[:, :], in_=ones_f[:, 0:128],
                pattern=[[-1, 128]], compare_op=ALU.is_equal, fill=0.0,
                base=0, channel_multiplier=1,
            )
            # band[n, l] = (n - l >= 0) & (n - l - 3 <= 0)
            nc.gpsimd.affine_select(
                out=band32[:, :], in_=ones_f[:, 0:128],
                pattern=[[-1, 128]], compare_op=ALU.is_ge, fill=0.0,
                base=0, channel_multiplier=1,
            )
            nc.gpsimd.affine_select(
                out=band32[:, :], in_=band32[:, :],
                pattern=[[1, 128]], compare_op=ALU.is_ge, fill=0.0,
                base=3, channel_multiplier=-1,
            )
            # bandb[r, l] = (l - 125 - r >= 0)
            nc.gpsimd.affine_select(
                out=bandb32[:, :], in_=ones_f[0:3, 0:128],
                pattern=[[1, 128]], compare_op=ALU.is_ge, fill=0.0,
                base=-125, channel_multiplier=-1,
            )
            nc.vector.tensor_copy(band[:, :], band32[:, :])
            nc.vector.tensor_copy(bandb[:, :], bandb32[:, :])

            # ---- d-major bf16 layout via fp32 PE transposes; the
            # psum->sbuf copy performs the bf16 cast (and the -2 scale for
            # x2's Gram operand).
            x_dn16 = {0: {}, 1: {}}
            for t in (0, 1):
                for b in range(BPC):
                    dn = work.tile(
                        [128, NP], bf16, tag=f"x{t}dn{b}", name=f"x{t}dn{b}"
                    )
                    tpp = tp_ps.tile([128, N], f32, tag="tp")
                    for c in (0, 1):
                        nc.tensor.transpose(
                            tpp[:, c * 128 : (c + 1) * 128],
                            x_nd[t][:, b, c, :],
                            ident32[:, :],
                        )
                    nc.tensor.transpose(
                        tpp[:, 256:259], x_l[t][:, b, :], ident32[0:3, 0:3]
                    )
                    nc.vector.tensor_scalar(
                        out=dn[:, 0:N], in0=tpp[:, 0:N],
                        scalar1=1.0 if t == 0 else -2.0,
                        scalar2=None, op0=ALU.mult,
                    )
                    x_dn16[t][b] = dn

            # xsq[t][b] = x_t_dn16^2 (bf16) on the DVE (2x bf16 mode); x2's
            # dn is scaled by -2 so its square is 4*x2^2 — compensated by
            # using 0.25-valued "ones" in its broadcast matmul.
            xsq = {0: {}, 1: {}}
            for t in (0, 1):
                for b in range(BPC):
                    sq = work.tile(
                        [128, NP], bf16, tag=f"xsq{t}{b}", name=f"xsq{t}{b}"
                    )
                    nc.scalar.activation(
                        sq[:, 0:N], x_dn16[t][b][:, 0:N], AF.Square,
                    )
                    xsq[t][b] = sq

            # ---- per-chunk sq2 + EPS (sqrt bias) ----
            a_cols = {0: {}, 1: {}}  # (128, 4) sbuf; cols = chunks
            for t in (0, 1):
                for b in range(BPC):
                    a_cols[t][b] = work.tile(
                        [128, 4], f32, tag=f"a{t}c{b}", name=f"a{t}c{b}"
                    )

            # wx tiles (weighted inputs for the pooling matmul); c-slot 2
            # rows 0..2 hold the weighted leftover rows n=256..258
            wx = {}
            for t in (0, 1):
                wx[t] = work.tile(
                    [128, BPC, 3, H], f32r, tag=f"wx{t}", name=f"wx{t}"
                )

            def emit_wx(t, b):
                for c in (0, 1):
                    nc.vector.tensor_scalar(
                        out=wx[t][:, b, c, :], in0=x_nd[t][:, b, c, :],
                        scalar1=a_cols[t][b][:, c : c + 1],
                        scalar2=None, op0=ALU.mult,
                    )
                nc.vector.tensor_scalar(
                    out=wx[t][0:3, b, 2, :], in0=x_l[t][:, b, :],
                    scalar1=a_cols[t][b][0:3, 2:3],
                    scalar2=None, op0=ALU.mult,
                )

            # ---- attention chunks ----
            x1row_sb = {}
            for b in range(BPC):
                rowp = row_ps.tile([1, NP], f32, tag="x1row")
                for ci, (i0, P) in enumerate(CHUNKS):
                    g = gram_ps.tile([128, NP], f32, tag="gram")
                    # -2 * x2[:,i] . x1[:,j]
                    nc.tensor.matmul(
                        g[0:P, :],
                        x_dn16[1][b][:, i0 : i0 + P],
                        x_dn16[0][b][:, :],
                        start=True, stop=False,
                    )
                    # + sq1[j] broadcast over i  (ones^T @ xsq1)
                    nc.tensor.matmul(
                        g[0:P, :],
                        ones16[:, 0:P],
                        xsq[0][b][:, :],
                        start=False, stop=False,
                    )
                    # + sq2[i] broadcast over j: xsq2 holds (2*x2)^2, the
                    # 0.25-valued moving operand recovers x2^2.
                    nc.tensor.matmul(
                        g[0:P, :],
                        xsq[1][b][:, i0 : i0 + P],
                        quart16[:, :],
                        start=False, stop=True,
                    )
                    # e = sqrt(psum + EPS)
                    e = epool.tile([128, NP], f32, tag="e")
                    nc.scalar.activation(
                        e[0:P, :], g[0:P, :], AF.Sqrt,
                        bias=epsb[0:P, 0:1],
                    )
                    # att = 1/(1+e) approx; accum -> x2_a column
                    att = attpool.tile([128, NP], bf16, tag="att")
                    nc.vector._custom_dve(
                        RECIP_OP,
                        out=att[0:P, 0:N], in0=e[0:P, 0:N],
                        s0=RECIP_C0, s1=RECIP_C1,
                        accum_out=a_cols[1][b][0:P, ci : ci + 1],
                    )
                    # x1_a row accumulation: ones_col^T @ att
                    nc.tensor.matmul(
                        rowp[:, :],
                        ones16[0:P, 0:1],
                        att[0:P, :],
                        start=(ci == 0), stop=(ci == 2),
                    )
                row_sb = work.tile([1, NP], f32, tag="x1row_sb", name=f"x1row{b}")
                nc.vector.tensor_copy(row_sb[:, :], rowp[:, :])
                x1row_sb[b] = row_sb
                # x2 weights for this batch are complete: start wx2 now
                emit_wx(1, b)

            # x1_a row -> per-partition columns via tiny K=1 matmuls
            for b in range(BPC):
                ac = acol_ps.tile([128, 4], f32, tag="acolp")
                for ci, (i0, P) in enumerate(CHUNKS):
                    nc.tensor.matmul(
                        ac[0:P, ci : ci + 1],
                        x1row_sb[b][:, i0 : i0 + P],
                        ones32[0:1, 0:1],
                        start=True, stop=True,
                    )
                nc.vector.tensor_copy(a_cols[0][b][:, :], ac[:, :])
                emit_wx(0, b)

            # ---- weighted sliding-window pooling via banded matmul (f32r),
            # batch-split so batch 0's output drains while batch 1 computes ----
            for b in range(BPC):
                for t in (0, 1):
                    bp = band_ps.tile([128, 2, BPC, H], f32, tag="bandp")
                    # main rows: out[l, lc, h] += band[n, l] * wx[n, b, lc, h]
                    nc.tensor.matmul(
                        bp[:, :, b, :],
                        band[:, :],
                        wx[t][:, b, 0:2, :],
                        start=True, stop=False,
                    )
                    # boundary: lc=0 <- n=128..130 (c=1), lc=1 <- n=256..258
                    nc.tensor.matmul(
                        bp[:, :, b, :],
                        bandb[0:3, :],
                        wx[t][0:3, b, 1:3, :],
                        start=False, stop=True,
                    )
                    osb = work.tile(
                        [128, 2, H], f32, tag=f"osb{t}{b}", name=f"osb{t}{b}"
                    )
                    if t == 0:
                        nc.scalar.copy(osb[:, :, :], bp[:, :, b, :])
                    else:
                        nc.vector.tensor_copy(osb[:, :, :], bp[:, :, b, :])
                    (nc.sync if t == 0 else nc.scalar).dma_start(
                        out=outd[t][b, 0, :, :].rearrange(
                            "(lc p) h -> p lc h", lc=2
                        ),
                        in_=osb[:, :, :],
                    )
    # TRN2 allows at most 1 sem wait per instruction (2 on EventSemaphore);
    # Tile can attach more — split them like Bacc.compile does, then encode
    # InstISA subclasses (custom DVE ops) to raw ISA bytes.
    import bass_rust
    from concourse import mybir as _mybir
    bass_rust.generate_event_semaphores(nc)
    _mybir.codegen_inst_isa_subclasses(nc)
    return nc


_NC_CACHE = {}


def _get_nc():
    if "nc" not in _NC_CACHE:
        _NC_CACHE["nc"] = build_nc()
    return _NC_CACHE["nc"]


def _run(x1, x2, **kwargs):
    x1 = np.ascontiguousarray(np.asarray(x1), dtype=np.float32)
    x2 = np.ascontiguousarray(np.asarray(x2), dtype=np.float32)
    nc = _get_nc()
    core_ids = list(range(NCORES))
    in_maps = [
        {
            "x1": x1[c * BPC : (c + 1) * BPC],
            "x2": x2[c * BPC : (c + 1) * BPC],
        }
        for c in core_ids
    ]
    br = run_bass_kernel_spmd(nc, in_maps, core_ids, **kwargs)
    out1 = np.concatenate([r["out1"] for r in br.results], axis=0)
    out2 = np.concatenate([r["out2"] for r in br.results], axis=0)
    return (out1, out2), br


def kernel(x1, x2):
    (out1, out2), _ = _run(x1, x2)
    return (out1, out2)


if __name__ == "__main__":
    rng = np.random.default_rng(0)
    x1 = rng.standard_normal((B, 1, N, H)).astype(np.float32)
    x2 = rng.standard_normal((B, 1, N, H)).astype(np.float32)
    o1, o2 = kernel(x1, x2)
    print("out shapes:", o1.shape, o2.shape)


# revision 38
# speedup vs baseline: 1.2388x; 1.0135x over previous
"""Trainium2 Bass kernel for nn_Abcnn2Portion (ABCNN-2 attention pooling).

Shapes (hardcoded): B=16, N=259 (L=256 + W-1=3), H=128, W=4, EPS=1e-6.
Reference:
    att[b,i,j] = 1 / (1 + sqrt(||x1[b,0,j,:] - x2[b,0,i,:]||^2 + EPS))
    x1_a[b,j] = sum_i att[b,i,j];  x2_a[b,i] = sum_j att[b,i,j]
    out_t[b,0,l,:] = sum_{k=0..3} x_t[b,0,l+k,:] * a_t[b,l+k],  l in [0,256)
Returns (out1, out2), each (16,1,256,128) fp32.

Strategy: data-parallel over batch, 2 batches per core across 8 cores.
Per batch:
  - pairwise squared distances via PE matmuls accumulated in PSUM
    (s2 = -2*x2.x1 + sq1[j] + sq2[i], all via bf16 matmuls; the d-major
    layout comes from fp32 PE transposes whose psum->sbuf copy casts to
    bf16 and folds the -2 scale for x2);
  - sqrt on the scalar engine (EPS via a per-partition bias tile);
  - att = 1/(1+e) fused with the row-sum in a single custom DVE op
    (bit-trick seed + one Newton step, zero-mean ~0.17% elementwise,
    averages out in the sums);
  - column sums via ones-matmul;
  - the W=4 sliding-window weighted pooling as a banded-matrix matmul
    (float32r), batch-split so each batch's output DMA drains while the
    other batch still computes.

Measured on trn2 (8 cores): ~34 us NEFF execution, rel err ~1.0e-3 vs
the fp32 jax reference. Roughly 7 us of that is fixed NRT preamble
(entry barrier + per-engine IRAM loads) and ~10 us the Tile kernel-tail
drain/barrier/sem-clear sequence; the PE on this terminal is capped at
1.2 GHz (power profile), which sets the matmul costs.
"""

import numpy as np

import concourse.bass as bass
import concourse.tile as tile
from concourse import mybir
from concourse.bass_utils import run_bass_kernel_spmd

# --------------------------------------------------------------------------
# Custom DVE op: out = approx(1/(1 + x)), accum_out = sum(out, free axis).
# --------------------------------------------------------------------------
import concourse.dve_ops as dve_ops
from concourse.dve_spec import Spec, Src0, C0, C1, One, AluOp, Bin, lower, _has_src1
from concourse.dve_ops import DveOp, OPS
from concourse.dve_uop import DveOpSpec

_S = Src0 + One
_nt = Bin(AluOp.BITWISE_NOT, _S, _S)
_y0 = _nt * C0
_BODY = _y0 * (C1 - _S * _y0)


def _recip_ref(in0, in1, s0, s1, imm2):
    S = (in0.astype(np.float32) + np.float32(1.0)).astype(np.float32)
    nt = (~S.view(np.int32)).view(np.float32)
    y0 = nt * np.float32(s0)
    return y0 * (np.float32(s1) - S * y0)


def _register_recip_op():
    name = "ADD1_RECIP_SUM_ANT"
    for existing in OPS:
        if existing.name == name:
            return existing
    spec = Spec(body=_BODY, accum=AluOp.ADD, reference=_recip_ref)
    op = DveOp(name, spec, subdim=False, uops_sha={})
    OPS.append(op)
    dve_ops._SUB_OPCODE_FOR_NAME[name] = dve_ops._CUSTOM_DVE_ROW_BASE + len(OPS) - 1
    for ver in ("v3", "v4"):
        op.uops_sha[ver] = DveOpSpec(
            name=name,
            opcode=dve_ops.get_dve_sub_opcode(name),
            uops=lower(spec, ver=ver),
            rd1_en=_has_src1(spec),
        ).sha(ver)
    return op


RECIP_OP = _register_recip_op()
RECIP_C0 = -0.23549792
RECIP_C1 = 2.0017324

# --------------------------------------------------------------------------
# Problem constants
# --------------------------------------------------------------------------
B, L, W, H = 16, 256, 4, 128
N = L + W - 1  # 259
EPS = 1e-6
NCORES = 8
BPC = B // NCORES  # batches per core = 2

f32 = mybir.dt.float32
f32r = mybir.dt.float32r
bf16 = mybir.dt.bfloat16
AF = mybir.ActivationFunctionType
ALU = mybir.AluOpType

# i-chunk decomposition of N=259: two full 128-partition chunks + 3 leftover
CHUNKS = [(0, 128), (128, 128), (256, 3)]
# pad the shared free dim to keep matmul moving dims even (f32r requires it;
# harmless for bf16). The pad column is garbage and excluded where it matters.
NP = 260


def _host_consts():
    """DMA-able constant tables: identity, band main, band boundary."""
    ident = np.eye(128, dtype=np.float32)
    band = np.zeros((128, 128), dtype=np.float32)  # band[n, l] = 1 if l<=n<=l+3
    for n in range(128):
        lo = max(0, n - (W - 1))
        band[n, lo : n + 1] = 1.0
    bandb = np.zeros((128, 128), dtype=np.float32)
    for r in range(W - 1):  # boundary rows n = 128+r relative to l in [0,128)
        bandb[r, 125 + r : 128] = 1.0
    return np.stack([ident, band, bandb])  # (3, 128, 128)


def build_nc():
    nc = bass.Bass()
    _orig_dab = tile.TileContext._drain_and_barrier

    def _light_dab(self, tick_clock, wait_clock):
        import bass_rust as _br
        _vc_mod = __import__('concourse.vector_clock', fromlist=['ScopedClock'])
        drain_inst = self.nc.sync.drain()
        gvc = tick_clock.global_clock
        dvc = _br.VectorClock([0] * _br.N_PROCS)
        for p in range(11, _br.N_PROCS):  # DMASW0..7, DMAHW0..7
            t = gvc[p]
            if t > 0:
                dvc.require_at_least(p, t)
        wait_clock.add_sem_waits(
            drain_inst.ins, _vc_mod.ScopedClock({None: dvc})
        )
        self.nc.all_engine_barrier(sem_only=True)
        assert self.sems is not None
        popped = self.nc._tile_sem_poison_stack.pop()
        assert popped is self._sem_poison
        self.nc.clear_and_free_semaphores(list(self.sems.allocated().values()))
        self.nc.all_engine_barrier(sem_only=True)

    tile.TileContext._drain_and_barrier = _light_dab
    try:
        _build_body(nc)
    finally:
        tile.TileContext._drain_and_barrier = _orig_dab
    # TRN2 allows at most 1 sem wait per instruction (2 on EventSemaphore);
    # Tile can attach more — split them like Bacc.compile does, then encode
    # InstISA subclasses (custom DVE ops) to raw ISA bytes.
    import bass_rust
    from concourse import mybir as _mybir
    bass_rust.generate_event_semaphores(nc)
    _mybir.codegen_inst_isa_subclasses(nc)
    return nc


def _build_body(nc):
    x1_in = nc.dram_tensor("x1", [BPC, 1, N, H], f32, kind="ExternalInput")
    x2_in = nc.dram_tensor("x2", [BPC, 1, N, H], f32, kind="ExternalInput")
    out1_d = nc.dram_tensor("out1", [BPC, 1, L, H], f32, kind="ExternalOutput")
    out2_d = nc.dram_tensor("out2", [BPC, 1, L, H], f32, kind="ExternalOutput")

    xin = {0: x1_in, 1: x2_in}
    outd = {0: out1_d, 1: out2_d}

    with tile.TileContext(nc) as tc:
        with (
            tc.tile_pool(name="singles", bufs=1) as singles,
            tc.tile_pool(name="work", bufs=1) as work,
            tc.tile_pool(name="epool", bufs=6) as epool,
            tc.tile_pool(name="attpool", bufs=6) as attpool,
            tc.tile_pool(name="tp_ps", bufs=2, space="PSUM") as tp_ps,
            tc.tile_pool(name="gram_ps", bufs=4, space="PSUM") as gram_ps,
            tc.tile_pool(name="row_ps", bufs=1, space="PSUM") as row_ps,
            tc.tile_pool(name="acol_ps", bufs=1, space="PSUM") as acol_ps,
        ):
            # ---- dependency-free startup: act-table trigger ----
            epsb = singles.tile([128, 1], f32, tag="epsb")
            escr = singles.tile([128, 1], f32, tag="escr")
            nc.vector.memset(epsb[:, :], EPS)
            # tiny Sqrt with no data deps: starts the 2.7us ACT table load
            # (sqrt_and_others, which also contains Copy/Identity/Square)
            # at t~0 instead of mid-kernel.
            nc.scalar.activation(escr[:, :], epsb[:, :], AF.Sqrt)

            # ---- inputs first: big x loads on the two HWDGE rings ----
            x_nd, x_l = {}, {}
            for t in (0, 1):
                x_nd[t] = work.tile(
                    [128, BPC, 2, H], f32, tag=f"x{t}nd", name=f"x{t}nd"
                )
                x_l[t] = work.tile([3, BPC, H], f32, tag=f"x{t}l", name=f"x{t}l")
                dma_eng = nc.sync if t == 0 else nc.scalar
                for b in range(BPC):
                    dma_eng.dma_start(
                        out=x_nd[t][:, b, :, :],
                        in_=xin[t][b, 0, 0:L, :].rearrange(
                            "(c p) h -> p c h", c=2
                        ),
                    )
            # small leftover rows on the sync ring (keep the scalar/ACT
            # queue free for activations)
            for t in (0, 1):
                for b in range(BPC):
                    nc.sync.dma_start(
                        out=x_l[t][:, b, :], in_=xin[t][b, 0, L:N, :]
                    )

            # ---- constants, generated on-chip (no DMA) ----
            band32 = singles.tile([128, 128], f32, tag="band32")
            bandb32 = singles.tile([3, 128], f32, tag="bandb32")
            ident32 = singles.tile([128, 128], f32, tag="ident32")
            band = singles.tile([128, 128], f32r, tag="band")
            bandb = singles.tile([3, 128], f32r, tag="bandb")
            ones16 = singles.tile([128, NP], bf16, tag="ones16")
            ones_f = singles.tile([128, NP], f32, tag="ones_f")
            ones32 = singles.tile([1, 1], f32, tag="ones32")
            nc.gpsimd.memset(ones_f[:, :], 1.0)
            nc.vector.tensor_copy(ones16[:, :], ones_f[:, :])
            quart16 = singles.tile([128, NP], bf16, tag="quart16")
            nc.vector.tensor_scalar(
                out=quart16[:, :], in0=ones_f[:, :],
                scalar1=0.25, scalar2=None, op0=ALU.mult,
            )
            nc.gpsimd.memset(ones32[:, :], 1.0)
            # ident[p, l] = (p - l == 0)
            nc.gpsimd.affine_select(
                out=ident32[:, :], in_=ones_f[:, 0:128],
                pattern=[[-1, 128]], compare_op=ALU.is_equal, fill=0.0,
                base=0, channel_multiplier=1,
            )
            # band[n, l] = (n - l >= 0) & (n - l - 3 <= 0)
            nc.gpsimd.affine_select(
                out=band32[:, :], in_=ones_f[:, 0:128],
                pattern=[[-1, 128]], compare_op=ALU.is_ge, fill=0.0,
                base=0, channel_multiplier=1,
            )
            nc.gpsimd.affine_select(
                out=band32[:, :], in_=band32[:, :],
                pattern=[[1, 128]], compare_op=ALU.is_ge, fill=0.0,
                base=3, channel_multiplier=-1,
            )
            # bandb[r, l] = (l - 125 - r >= 0)
            nc.gpsimd.affine_select(
                out=bandb32[:, :], in_=ones_f[0:3, 0:128],
                pattern=[[1, 128]], compare_op=ALU.is_ge, fill=0.0,
                base=-125, channel_multiplier=-1,
            )
            nc.vector.tensor_copy(band[:, :], band32[:, :])
            nc.vector.tensor_copy(bandb[:, :], bandb32[:, :])

            # ---- d-major bf16 layout via fp32 PE transposes; the
            # psum->sbuf copy performs the bf16 cast (and the -2 scale for
            # x2's Gram operand).
            x_dn16 = {0: {}, 1: {}}
            for t in (0, 1):
                for b in range(BPC):
                    dn = work.tile(
                        [128, NP], bf16, tag=f"x{t}dn{b}", name=f"x{t}dn{b}"
                    )
                    tpp = tp_ps.tile([128, N], f32, tag="tp")
                    for c in (0, 1):
                        nc.tensor.transpose(
                            tpp[:, c * 128 : (c + 1) * 128],
                            x_nd[t][:, b, c, :],
                            ident32[:, :],
                        )
                    nc.tensor.transpose(
                        tpp[:, 256:259], x_l[t][:, b, :], ident32[0:3, 0:3]
                    )
                    nc.vector.tensor_scalar(
                        out=dn[:, 0:N], in0=tpp[:, 0:N],
                        scalar1=1.0 if t == 0 else -2.0,
                        scalar2=None, op0=ALU.mult,
                    )
                    x_dn16[t][b] = dn

            # xsq[t][b] = x_t_dn16^2 (bf16) on the DVE (2x bf16 mode); x2's
            # dn is scaled by -2 so its square is 4*x2^2 — compensated by
            # using 0.25-valued "ones" in its broadcast matmul.
            xsq = {0: {}, 1: {}}
            for t in (0, 1):
                for b in range(BPC):
                    sq = work.tile(
                        [128, NP], bf16, tag=f"xsq{t}{b}", name=f"xsq{t}{b}"
                    )
                    nc.scalar.activation(
                        sq[:, 0:N], x_dn16[t][b][:, 0:N], AF.Square,
                    )
                    xsq[t][b] = sq

            # ---- per-chunk sq2 + EPS (sqrt bias) ----
            a_cols = {0: {}, 1: {}}  # (128, 4) sbuf; cols = chunks
            for t in (0, 1):
                for b in range(BPC):
                    a_cols[t][b] = work.tile(
                        [128, 4], f32, tag=f"a{t}c{b}", name=f"a{t}c{b}"
                    )

            # wx tiles (weighted inputs for the pooling matmul); c-slot 2
            # rows 0..2 hold the weighted leftover rows n=256..258
            wx = {}
            for t in (0, 1):
                wx[t] = work.tile(
                    [128, BPC, 3, H], f32r, tag=f"wx{t}", name=f"wx{t}"
                )

            def emit_wx(t, b):
                for c in (0, 1):
                    nc.vector.tensor_scalar(
                        out=wx[t][:, b, c, :], in0=x_nd[t][:, b, c, :],
                        scalar1=a_cols[t][b][:, c : c + 1],
                        scalar2=None, op0=ALU.mult,
                    )
                nc.vector.tensor_scalar(
                    out=wx[t][0:3, b, 2, :], in0=x_l[t][:, b, :],
                    scalar1=a_cols[t][b][0:3, 2:3],
                    scalar2=None, op0=ALU.mult,
                )

            # ---- attention chunks ----
            x1row_sb = {}
            for b in range(BPC):
                rowp = row_ps.tile([1, NP], f32, tag="x1row")
                for ci, (i0, P) in enumerate(CHUNKS):
                    g = gram_ps.tile([128, NP], f32, tag="gram")
                    # -2 * x2[:,i] . x1[:,j]
                    nc.tensor.matmul(
                        g[0:P, :],
                        x_dn16[1][b][:, i0 : i0 + P],
                        x_dn16[0][b][:, :],
                        start=True, stop=False,
                    )
                    # + sq1[j] broadcast over i  (ones^T @ xsq1)
                    nc.tensor.matmul(
                        g[0:P, :],
                        ones16[:, 0:P],
                        xsq[0][b][:, :],
                        start=False, stop=False,
                    )
                    # + sq2[i] broadcast over j: xsq2 holds (2*x2)^2, the
                    # 0.25-valued moving operand recovers x2^2.
                    nc.tensor.matmul(
                        g[0:P, :],
                        xsq[1][b][:, i0 : i0 + P],
                        quart16[:, :],
                        start=False, stop=True,
                    )
                    # e = sqrt(psum + EPS)
                    e = epool.tile([128, NP], f32, tag="e")
                    nc.scalar.activation(
                        e[0:P, :], g[0:P, :], AF.Sqrt,
                        bias=epsb[0:P, 0:1],
                    )
                    # att = 1/(1+e) approx; accum -> x2_a column
                    att = attpool.tile([128, NP], bf16, tag="att")
                    nc.vector._custom_dve(
                        RECIP_OP,
                        out=att[0:P, 0:N], in0=e[0:P, 0:N],
                        s0=RECIP_C0, s1=RECIP_C1,
                        accum_out=a_cols[1][b][0:P, ci : ci + 1],
                    )
                    # x1_a row accumulation: ones_col^T @ att
                    nc.tensor.matmul(
                        rowp[:, :],
                        ones16[0:P, 0:1],
                        att[0:P, :],
                        start=(ci == 0), stop=(ci == 2),
                    )
                row_sb = work.tile([1, NP], f32, tag="x1row_sb", name=f"x1row{b}")
                nc.vector.tensor_copy(row_sb[:, :], rowp[:, :])
                x1row_sb[b] = row_sb
                # x2 weights for this batch are complete: start wx2 now
                emit_wx(1, b)

            # x1_a row -> per-partition columns via tiny K=1 matmuls
            for b in range(BPC):
                ac = acol_ps.tile([128, 4], f32, tag="acolp")
                for ci, (i0, P) in enumerate(CHUNKS):
                    nc.tensor.matmul(
                        ac[0:P, ci : ci + 1],
                        x1row_sb[b][:, i0 : i0 + P],
                        ones32[0:1, 0:1],
                        start=True, stop=True,
                    )
                nc.vector.tensor_copy(a_cols[0][b][:, :], ac[:, :])
                emit_wx(0, b)

            # ---- weighted sliding-window pooling via banded matmul (f32r),
            # batch-split so batch 0's output drains while batch 1 computes ----
            for b in range(BPC):
                for t in (0, 1):
                    bp = tp_ps.tile([128, 2, BPC, H], f32, tag="tp", name="bp")
                    # main rows: out[l, lc, h] += band[n, l] * wx[n, b, lc, h]
                    nc.tensor.matmul(
                        bp[:, :, b, :],
                        band[:, :],
                        wx[t][:, b, 0:2, :],
                        start=True, stop=False,
                    )
                    # boundary: lc=0 <- n=128..130 (c=1), lc=1 <- n=256..258
                    nc.tensor.matmul(
                        bp[:, :, b, :],
                        bandb[0:3, :],
                        wx[t][0:3, b, 1:3, :],
                        start=False, stop=True,
                    )
                    osb = work.tile(
                        [128, 2, H], f32, tag=f"osb{t}{b}", name=f"osb{t}{b}"
                    )
                    if t == 0:
                        nc.scalar.copy(osb[:, :, :], bp[:, :, b, :])
                    else:
                        nc.vector.tensor_copy(osb[:, :, :], bp[:, :, b, :])
                    (nc.sync if t == 0 else nc.scalar).dma_start(
                        out=outd[t][b, 0, :, :].rearrange(
                            "(lc p) h -> p lc h", lc=2
                        ),
                        in_=osb[:, :, :],
                    )
    # TRN2 allows at most 1 sem wait per instruction (2 on EventSemaphore);
    # Tile can attach more — split them like Bacc.compile does, then encode
    # InstISA subclasses (custom DVE ops) to raw ISA bytes.
    import bass_rust
    from concourse import mybir as _mybir
    bass_rust.generate_event_semaphores(nc)
    _mybir.codegen_inst_isa_subclasses(nc)
    return nc


_NC_CACHE = {}


def _get_nc():
    if "nc" not in _NC_CACHE:
        _NC_CACHE["nc"] = build_nc()
    return _NC_CACHE["nc"]


def _run(x1, x2, **kwargs):
    x1 = np.ascontiguousarray(np.asarray(x1), dtype=np.float32)
    x2 = np.ascontiguousarray(np.asarray(x2), dtype=np.float32)
    nc = _get_nc()
    core_ids = list(range(NCORES))
    in_maps = [
        {
            "x1": x1[c * BPC : (c + 1) * BPC],
            "x2": x2[c * BPC : (c + 1) * BPC],
        }
        for c in core_ids
    ]
    br = run_bass_kernel_spmd(nc, in_maps, core_ids, **kwargs)
    out1 = np.concatenate([r["out1"] for r in br.results], axis=0)
    out2 = np.concatenate([r["out2"] for r in br.results], axis=0)
    return (out1, out2), br


def kernel(x1, x2):
    (out1, out2), _ = _run(x1, x2)
    return (out1, out2)


if __name__ == "__main__":
    rng = np.random.default_rng(0)
    x1 = rng.standard_normal((B, 1, N, H)).astype(np.float32)
    x2 = rng.standard_normal((B, 1, N, H)).astype(np.float32)
    o1, o2 = kernel(x1, x2)
    print("out shapes:", o1.shape, o2.shape)
